# revision 1
# baseline (speedup 1.0000x reference)
"""Hetero GNN encoder/decoder (SAGE x2 + BN + edge MLP decoder) on 8 trn2 cores.

Strategy (edge sharding by destination, node-range sharding):
  - Articles: core k owns rows [k*APC, (k+1)*APC); customers likewise (CPC).
  - Message edges partitioned by dst-owner core; mean-aggregation is computed
    fully locally (scale 1/cnt folded into the one-hot), via
    dma_gather (int16 per-src-block indices) + one-hot matmul (X^T @ P)
    scatter into PSUM windows of 128 nodes.  One-hot P tiles are built in
    batches of 8 tiles with wide tensor_tensor ops (DVE op overhead
    dominates, and GpSimd SWDGE descriptor-gen contends for the shared
    DVE/POOL SBUF port - fewer, wider DVE ops).
  - After each SAGE layer, node features are AllGathered (row-major fp16
    tables) so the next layer / decoder can gather from the full table.
  - BatchNorm: local per-channel partial sums + tiny AllReduce.
  - Decoder uses precomputed U_c = bn(z_c) @ Wd1[:128] + b_dec1 and
    U_a = bn(z_a) @ Wd1[128:] tables; per label y = w2 . relu(U_c[lc]+U_a[la])
    + b2 - no PE work in the decoder loop at all.

All structure (loop bounds, window emissions) is compile-time and identical
across cores; per-core variation lives in the data (padded to uniform sizes).
"""
import sys

sys.path.insert(0, "/opt/trn_rl_repo")

import numpy as np

import concourse.bacc as bacc
import concourse.bass as bass
import concourse.mybir as mybir
import concourse.tile as tile
from concourse.bass_utils import run_bass_kernel_spmd
from concourse.masks import make_identity

P = 128
NCORES = 8
GCH = 1024          # indices per dma_gather
TPC = GCH // P      # tiles per gather chunk (8)
WCH = 512           # W-stage column chunk
MAXW = 4            # max windows per tile (window-relative encoding)
BN_EPS = 1e-5


class Cfg:
    def __init__(self, n_c=300000, n_a=100000, e_lbl=1000000,
                 sbn=1280, srcb_c=30000, srcb_a=25000, f16=True):
        self.n_c, self.n_a, self.e_lbl = n_c, n_a, e_lbl
        self.cpc, self.apc = n_c // NCORES, n_a // NCORES
        assert self.cpc * NCORES == n_c and self.apc * NCORES == n_a
        self.chalf = self.cpc // 2
        assert self.chalf * 2 == self.cpc
        self.sbn = sbn
        self.srcb_c, self.srcb_a = srcb_c, srcb_a
        self.nblk_c = -(-n_c // srcb_c)
        self.nblk_a = -(-n_a // srcb_a)
        assert srcb_c < 32768 and srcb_a < 32768
        self.zc_sub = -(-self.cpc // 2)          # U_c local gather sub-block
        assert self.zc_sub < 32768
        self.f16 = f16
        self.dt = mybir.dt.float16 if f16 else mybir.dt.float32
        self.npdt = np.float16 if f16 else np.float32


def _ru(x, m):
    return (x + m - 1) // m * m


def _wrap_idx_chunk(flat):
    """[n] int16 -> [128, n/16] wrap (16-partition, replicated x8)."""
    n = flat.shape[0]
    w = flat.astype(np.int16).reshape(n // 16, 16).T
    return np.tile(w, (8, 1))


def _pack_pcol(a):
    """[n] -> [128, n/128]: element i -> partition i%128, col i//128."""
    return np.ascontiguousarray(a.reshape(-1, P).T)


# ---------------------------------------------------------------------------
# host-side structure + array prep for one aggregation pass
# ---------------------------------------------------------------------------

class AggPass:
    """Static structure (shared across cores) + per-core packed arrays."""

    def __init__(self, name, nloc, srcb, nsrc_blk, nsrc_rows, sbn):
        self.name = name
        self.nloc = nloc
        self.srcb = srcb
        self.nsrc_blk = nsrc_blk
        self.nsrc_rows = nsrc_rows
        self.sbn = sbn
        self.nsb = -(-nloc // sbn)
        self.sb_nodes = [min(sbn, nloc - s * sbn) for s in range(self.nsb)]
        self.run_L = None        # [nsb, nsrc_blk] uniform padded run lengths
        self.etot = 0
        self.emits = None        # per sb: list of (j, t, wlo, nwin) per tile
        self.uncovered = None    # per sb: list of never-touched windows
        self.idx = None          # per core [128, etot/16] int16
        self.dsc = None          # per core [128, 2*etot/128] f16 (dst|scl/run)


def prep_agg_pass(name, src, dst_loc, scale_e, core_e, nloc, srcb, nsrc_blk,
                  nsrc_rows, sbn):
    ap = AggPass(name, nloc, srcb, nsrc_blk, nsrc_rows, sbn)
    nsb = ap.nsb
    nruns = nsb * nsrc_blk

    per_core = []
    counts = np.zeros((NCORES, nruns), np.int64)
    for k in range(NCORES):
        m = core_e == k
        s, d, sc = src[m], dst_loc[m], scale_e[m]
        j = s // srcb
        sb = d // sbn
        order = np.lexsort((d, j, sb))
        s, d, sc, j, sb = s[order], d[order], sc[order], j[order], sb[order]
        rid = sb * nsrc_blk + j
        counts[k] = np.bincount(rid, minlength=nruns)
        per_core.append((s, d, sc, rid))

    run_L = _ru(counts.max(axis=0), P)          # uniform, 128-multiple
    offs = np.concatenate([[0], np.cumsum(run_L)]).astype(np.int64)
    etot = int(offs[-1])
    ap.run_L = run_L.reshape(nsb, nsrc_blk)
    ap.etot = etot

    # superblock-relative dst per core (for window structure)
    dstrel_all = np.full((NCORES, etot), -1.0e9, np.float64)
    pos_all = []
    for k in range(NCORES):
        s, d, sc, rid = per_core[k]
        run_start = np.concatenate([[0], np.cumsum(counts[k])])[:-1]
        pos = offs[rid] + (np.arange(len(s)) - run_start[rid])
        pos_all.append(pos)
        dstrel_all[k, pos] = (d - (d // sbn) * sbn).astype(np.float64)

    # per-tile window range (union over cores)
    T = etot // P
    Dw = dstrel_all.reshape(NCORES, T, P)
    valid_any = Dw.max(axis=2) >= 0              # [NCORES, T]
    wlo_c = np.where(Dw >= 0, Dw, np.inf).min(axis=2) // P
    whi_c = np.where(Dw >= 0, Dw, -np.inf).max(axis=2) // P
    wlo_t = np.where(valid_any, wlo_c, np.inf).min(axis=0)
    whi_t = np.where(valid_any, whi_c, -np.inf).max(axis=0)

    emits = []
    uncovered = []
    tile_wlo = np.zeros(T, np.int64)             # per global tile
    for s in range(nsb):
        nwin = -(-ap.sb_nodes[s] // P)
        covered = set()
        sb_emits = []
        for j in range(nsrc_blk):
            r = s * nsrc_blk + j
            o = int(offs[r])
            nt = int(ap.run_L[s, j] // P)
            for t in range(nt):
                g = o // P + t
                if np.isfinite(wlo_t[g]):
                    a = max(0, min(int(wlo_t[g]), nwin - 1))
                    b = max(a, min(int(whi_t[g]), nwin - 1))
                else:
                    a, b = 0, 0
                nw = b - a + 1
                assert nw <= MAXW, f"tile spans {nw} windows"
                covered.update(range(a, b + 1))
                tile_wlo[g] = a
                sb_emits.append((j, t, a, nw))
        emits.append(sb_emits)
        uncovered.append(sorted(set(range(nwin)) - covered))
    ap.emits = emits
    ap.uncovered = uncovered

    # per-core packed arrays (dst window-relative to tile_wlo)
    idxs, dscs = [], []
    for k in range(NCORES):
        s, d, sc, rid = per_core[k]
        pos = pos_all[k]
        idx16 = np.zeros(etot, np.int16)
        idx16[pos] = (s - (s // srcb) * srcb).astype(np.int16)
        dstrel = np.full(etot, -1000.0, np.float32)
        dstrel[pos] = (d - (d // sbn) * sbn).astype(np.float32)
        dstrel -= 128.0 * tile_wlo[np.arange(etot) // P]
        dstrel[dstrel < -1000.0] = -1000.0
        sclp = np.zeros(etot, np.float32)
        sclp[pos] = sc.astype(np.float32)

        wrapped = np.zeros((P, etot // 16), np.int16)
        dsc = np.zeros((P, 2 * (etot // P)), np.float16)
        for r in range(nruns):
            o = int(offs[r])
            L = int(run_L[r])
            if L == 0:
                continue
            wrapped[:, o // 16:(o + L) // 16] = _wrap_idx_chunk(idx16[o:o + L])
            nt = L // P
            c0 = 2 * (o // P)
            dsc[:, c0:c0 + nt] = _pack_pcol(dstrel[o:o + L]).astype(np.float16)
            dsc[:, c0 + nt:c0 + 2 * nt] = \
                _pack_pcol(sclp[o:o + L]).astype(np.float16)
        idxs.append(wrapped)
        dscs.append(dsc)
    ap.idx, ap.dsc = idxs, dscs
    return ap


# ---------------------------------------------------------------------------
# full host prep
# ---------------------------------------------------------------------------

def prep_all(cfg, inputs):
    i64 = lambda a: np.asarray(a).astype(np.int64)
    e_src = i64(inputs["edge_src_customer"])
    e_dst = i64(inputs["edge_dst_article"])
    l_c = i64(inputs["label_customer"])
    l_a = i64(inputs["label_article"])

    cnt_a = np.bincount(e_dst, minlength=cfg.n_a)
    cnt_c = np.bincount(e_src, minlength=cfg.n_c)
    scl_a = (1.0 / np.maximum(cnt_a, 1.0)).astype(np.float32)
    scl_c = (1.0 / np.maximum(cnt_c, 1.0)).astype(np.float32)

    pa = prep_agg_pass(
        "A", e_src, e_dst % cfg.apc, scl_a[e_dst], e_dst // cfg.apc,
        cfg.apc, cfg.srcb_c, cfg.nblk_c, cfg.n_c, cfg.sbn)
    dloc = e_src % cfg.cpc
    core_c = e_src // cfg.cpc
    half = (dloc >= cfg.chalf).astype(np.int64)
    pcs = []
    for h in range(2):
        m = half == h
        pcs.append(prep_agg_pass(
            f"C{h}", e_dst[m], dloc[m] - h * cfg.chalf, scl_c[e_src][m],
            core_c[m], cfg.chalf, cfg.srcb_a, cfg.nblk_a, cfg.n_a, cfg.sbn))

    # decoder labels: partition by customer core, group by (sub, ablk)
    core_l = l_c // cfg.cpc
    sub_l = (l_c % cfg.cpc) // cfg.zc_sub
    ablk_l = l_a // cfg.srcb_a
    gid = sub_l * cfg.nblk_a + ablk_l
    ngrp = 2 * cfg.nblk_a
    gcounts = np.zeros((NCORES, ngrp), np.int64)
    per_core_lbl = []
    for k in range(NCORES):
        m = core_l == k
        lc, la, g, orig = l_c[m], l_a[m], gid[m], np.nonzero(m)[0]
        order = np.argsort(g, kind="stable")
        lc, la, g, orig = lc[order], la[order], g[order], orig[order]
        gcounts[k] = np.bincount(g, minlength=ngrp)
        per_core_lbl.append((lc, la, g, orig))
    grp_L = _ru(gcounts.max(axis=0), P)
    goffs = np.concatenate([[0], np.cumsum(grp_L)]).astype(np.int64)
    ld_pad = int(goffs[-1])

    dec_idx_c, dec_idx_a, out_pos = [], [], []
    for k in range(NCORES):
        lc, la, g, orig = per_core_lbl[k]
        gstart = np.concatenate([[0], np.cumsum(gcounts[k])])[:-1]
        pos = goffs[g] + (np.arange(len(lc)) - gstart[g])
        ic = np.zeros(ld_pad, np.int16)
        ia = np.zeros(ld_pad, np.int16)
        po = np.full(ld_pad, -1, np.int64)
        lcl = lc % cfg.cpc
        ic[pos] = (lcl - (lcl // cfg.zc_sub) * cfg.zc_sub).astype(np.int16)
        ia[pos] = (la - (la // cfg.srcb_a) * cfg.srcb_a).astype(np.int16)
        po[pos] = orig
        wc = np.zeros((P, ld_pad // 16), np.int16)
        wa = np.zeros((P, ld_pad // 16), np.int16)
        for gi in range(ngrp):
            o = int(goffs[gi])
            L = int(grp_L[gi])
            if L:
                wc[:, o // 16:(o + L) // 16] = _wrap_idx_chunk(ic[o:o + L])
                wa[:, o // 16:(o + L) // 16] = _wrap_idx_chunk(ia[o:o + L])
        dec_idx_c.append(wc)
        dec_idx_a.append(wa)
        out_pos.append(po)

    dec = dict(grp_L=grp_L.reshape(2, cfg.nblk_a), goffs=goffs, ld_pad=ld_pad,
               idx_c=dec_idx_c, idx_a=dec_idx_a, out_pos=out_pos)
    return pa, pcs, dec


# ---------------------------------------------------------------------------
# kernel builder
# ---------------------------------------------------------------------------

F32 = mybir.dt.float32


def build_nc(cfg, pa, pcs, dec, dbg=False):
    DT = cfg.dt
    nc = bacc.Bacc("TRN2", target_bir_lowering=False, debug=False,
                   num_devices=NCORES, num_swdge_queues=4)
    qctr = [0]
    def next_q():
        qctr[0] = (qctr[0] + 1) % 4
        return qctr[0]

    ei = lambda n, s, d: nc.dram_tensor(n, s, d, kind="ExternalInput")
    xc = ei("xc", [cfg.n_c, P], DT)
    xa = ei("xa", [cfg.n_a, P], DT)
    xaT = ei("xaT", [P, cfg.apc], DT)
    xcT = ei("xcT", [P, cfg.cpc], DT)
    aggA_idx = ei("aggA_idx", [P, pa.etot // 16], mybir.dt.int16)
    aggA_dsc = ei("aggA_dsc", [P, 2 * pa.etot // P], DT)
    aggC_idx = [ei(f"aggC{h}_idx", [P, pcs[h].etot // 16], mybir.dt.int16)
                for h in range(2)]
    aggC_dsc = [ei(f"aggC{h}_dsc", [P, 2 * pcs[h].etot // P], DT)
                for h in range(2)]
    dec_idx_c = ei("dec_idx_c", [P, dec["ld_pad"] // 16], mybir.dt.int16)
    dec_idx_a = ei("dec_idx_a", [P, dec["ld_pad"] // 16], mybir.dt.int16)

    wnames = ["W_msg1_ca", "W_self1_a", "W_msg1_ac", "W_self1_c",
              "W_msg2_ca", "W_self2_a", "W_msg2_ac", "W_self2_c",
              "Wd1c", "Wd1a"]
    wts = {n: ei(n, [P, P], DT) for n in wnames}
    w2rep = ei("w2rep", [P, GCH], DT)       # W_dec2 repeated per 128-segment
    bnames = ["b1_a", "b1_c", "b2_a", "b2_c",
              "bn_gamma_c", "bn_beta_c", "bn_gamma_a", "bn_beta_a",
              "b_dec1", "b_dec2c"]
    bis = {n: ei(n, [P, 1], F32) for n in bnames}

    ldT = dec["ld_pad"] // P
    y_out = nc.dram_tensor("y", [P, ldT], F32, kind="ExternalOutput")
    dbg_outs = {}
    if dbg:
        dbg_outs = {
            "d_ha": nc.dram_tensor("d_ha", [cfg.n_a, P], DT,
                                   kind="ExternalOutput"),
            "d_hc": nc.dram_tensor("d_hc", [cfg.n_c, P], DT,
                                   kind="ExternalOutput"),
            "d_ua": nc.dram_tensor("d_ua", [cfg.n_a, P], DT,
                                   kind="ExternalOutput"),
            "d_uc": nc.dram_tensor("d_uc", [cfg.cpc, P], DT,
                                   kind="ExternalOutput"),
            "d_st": nc.dram_tensor("d_st", [P, 4], F32,
                                   kind="ExternalOutput"),
        }

    rg = [list(range(NCORES))]

    with tile.TileContext(nc) as tc:
        with (
            tc.tile_pool(name="dramp", bufs=1, space="DRAM") as dramp,
            tc.tile_pool(name="const", bufs=1) as cs,
        ):
            ha_own = dramp.tile([cfg.apc, P], DT)
            ha_full = dramp.tile([cfg.n_a, P], DT, addr_space="Shared")
            hc_own = dramp.tile([cfg.cpc, P], DT)
            hc_full = dramp.tile([cfg.n_c, P], DT, addr_space="Shared")
            ua_own = dramp.tile([cfg.apc, P], DT)
            ua_full = dramp.tile([cfg.n_a, P], DT, addr_space="Shared")
            uc_loc = dramp.tile([cfg.cpc, P], DT)
            haT_d = dramp.tile([P, cfg.apc], DT)
            hcT_d = dramp.tile([P, cfg.cpc], DT)
            zaT_d = dramp.tile([P, cfg.apc], DT)
            zcT_d = dramp.tile([P, cfg.cpc], DT)
            stats_in = dramp.tile([P, 4], F32)
            stats_out = dramp.tile([P, 4], F32, addr_space="Shared")

            # constants: batched iota tiles (values wi*128 + col%128)
            iota8 = []
            for wi in range(MAXW):
                ii = cs.tile([P, GCH], mybir.dt.int32, name=f"ioi{wi}")
                nc.gpsimd.iota(ii[:], pattern=[[0, TPC], [1, P]],
                               base=wi * P, channel_multiplier=0)
                ff = cs.tile([P, GCH], DT, name=f"iof{wi}")
                nc.vector.tensor_copy(ff[:], ii[:])
                iota8.append(ff)
            ident = cs.tile([P, P], DT)
            make_identity(nc, ident[:])
            w_sb = {n: cs.tile([P, P], DT, name=f"w_{n}") for n in wnames}
            for n in wnames:
                nc.sync.dma_start(out=w_sb[n][:], in_=wts[n][:])
            w2r_sb = cs.tile([P, GCH], DT)
            nc.sync.dma_start(out=w2r_sb[:], in_=w2rep[:])
            b_sb = {n: cs.tile([P, 1], F32, name=f"b_{n}") for n in bnames}
            for n in bnames:
                nc.sync.dma_start(out=b_sb[n][:], in_=bis[n][:])
            stats_sb = cs.tile([P, 4], F32)
            nc.vector.memset(stats_sb[:], 0.0)

            # ---------------- aggregation pass ----------------
            def agg_pass(ps, table, idx_d, dsc_d, meanT_sb):
                offs = np.concatenate(
                    [[0], np.cumsum(ps.run_L.reshape(-1))]).astype(np.int64)
                with (
                    tc.tile_pool(name=f"ag_{ps.name}", bufs=1) as sbp,
                    tc.tile_pool(name=f"agp_{ps.name}", bufs=2,
                                 space="PSUM") as psp,
                ):
                    for s in range(ps.nsb):
                        nodes = ps.sb_nodes[s]
                        nwin = -(-nodes // P)
                        psum = psp.tile([P, nwin * P], F32, tag="aggps",
                                        name="psum_agg", bufs=2)
                        # first/last touch per PSUM bank (4 windows/bank)
                        touch = {}
                        for (j, t, wlo, nw) in ps.emits[s]:
                            for wi in range(nw):
                                w = wlo + wi
                                touch.setdefault(w // 4, []).append((j, t, w))
                        firsts = {b: v[0] for b, v in touch.items()}
                        lasts = {b: v[-1] for b, v in touch.items()}

                        # group emits per run
                        by_run = {}
                        for e in ps.emits[s]:
                            by_run.setdefault(e[0], []).append(e)
                        for j in sorted(by_run):
                            r = s * ps.nsrc_blk + j
                            o = int(offs[r])
                            L = int(ps.run_L[s, j])
                            nt = L // P
                            blk_rows = min(ps.srcb,
                                           ps.nsrc_rows - j * ps.srcb)
                            idx_sb = sbp.tile([P, L // 16], mybir.dt.int16,
                                              tag="gidx", name="gidx",
                                              bufs=3)
                            nc.sync.dma_start(
                                out=idx_sb[:],
                                in_=idx_d[:, o // 16:(o + L) // 16])
                            dsc_sb = sbp.tile([P, 2 * nt], DT, tag="gdsc",
                                              name="gdsc", bufs=3)
                            nc.sync.dma_start(
                                out=dsc_sb[:],
                                in_=dsc_d[:, 2 * (o // P):2 * (o // P) + 2 * nt])
                            x_tiles = []
                            for c0 in range(0, L, GCH):
                                cl = min(GCH, L - c0)
                                x = sbp.tile([P, TPC, P], DT, tag="gx",
                                             name="gx", bufs=10)
                                nc.gpsimd.dma_gather(
                                    x[:, :cl // P, :],
                                    table[j * ps.srcb:
                                          j * ps.srcb + blk_rows, :],
                                    idx_sb[:, c0 // 16:(c0 + cl) // 16],
                                    cl, cl, P, queue_num=next_q())
                                x_tiles.append(x)
                            # per gather-chunk batched P builds
                            run_emits = by_run[j]
                            p8s = {}     # (chunk, wi) -> tile
                            for c in range(0, nt, TPC):
                                ctn = min(TPC, nt - c)
                                maxnw = max(e[3] for e in run_emits
                                            if c <= e[1] < c + ctn)
                                dstb = dsc_sb[:, c:c + ctn] \
                                    .to_broadcast([P, ctn, P])
                                sclb = dsc_sb[:, nt + c:nt + c + ctn] \
                                    .to_broadcast([P, ctn, P])
                                for wi in range(maxnw):
                                    p8 = sbp.tile([P, GCH], DT, tag="gp",
                                                  name="gp", bufs=6)
                                    p83 = p8[:, :ctn * P].rearrange(
                                        "p (t w) -> p t w", w=P)
                                    nc.vector.tensor_tensor(
                                        out=p83,
                                        in0=iota8[wi][:, :ctn * P].rearrange(
                                            "p (t w) -> p t w", w=P),
                                        in1=dstb,
                                        op=mybir.AluOpType.is_equal)
                                    nc.vector.tensor_tensor(
                                        out=p83, in0=p83, in1=sclb,
                                        op=mybir.AluOpType.mult)
                                    p8s[(c // TPC, wi)] = p8
                            for (j2, t, wlo, nw) in run_emits:
                                for wi in range(nw):
                                    w = wlo + wi
                                    p8 = p8s[(t // TPC, wi)]
                                    nc.tensor.matmul(
                                        psum[:, w * P:(w + 1) * P],
                                        lhsT=x_tiles[t // TPC]
                                            [:, t % TPC, :],
                                        rhs=p8[:, (t % TPC) * P:
                                               (t % TPC + 1) * P],
                                        start=(firsts[w // 4] == (j2, t, w)),
                                        stop=(lasts[w // 4] == (j2, t, w)),
                                        skip_group_check=True)
                        nc.vector.tensor_copy(
                            meanT_sb[:, s * ps.sbn:s * ps.sbn + nodes],
                            psum[:, :nodes])
                        for w in ps.uncovered[s]:
                            a = s * ps.sbn + w * P
                            b = min(a + P, s * ps.sbn + nodes)
                            nc.vector.memset(meanT_sb[:, a:b], 0.0)

            # ---------------- W stage ----------------
            def w_stage(nloc, meanT_sb, selfT_dram, self_off, wmsg, wself,
                        bias_col, relu, outT_dram, outT_off, rows_dram,
                        rows_off, stats_cols, sbp, psp):
                for c0 in range(0, nloc, WCH):
                    cw = min(WCH, nloc - c0)
                    sT = sbp.tile([P, WCH], DT, tag="wself", name="wselfT",
                                  bufs=3)
                    nc.sync.dma_start(
                        out=sT[:, :cw],
                        in_=selfT_dram[:, self_off + c0:self_off + c0 + cw])
                    psum = psp.tile([P, WCH], F32, tag="wps", name="wps",
                                    bufs=3)
                    nc.tensor.matmul(psum[:, :cw], lhsT=wmsg,
                                     rhs=meanT_sb[:, c0:c0 + cw],
                                     start=True, stop=False,
                                     skip_group_check=True)
                    nc.tensor.matmul(psum[:, :cw], lhsT=wself,
                                     rhs=sT[:, :cw],
                                     start=False, stop=True,
                                     skip_group_check=True)
                    oT = sbp.tile([P, WCH], DT, tag="woT", name="woT", bufs=3)
                    nc.scalar.activation(
                        oT[:, :cw], psum[:, :cw],
                        mybir.ActivationFunctionType.Relu if relu
                        else mybir.ActivationFunctionType.Identity,
                        bias=bias_col[:], scale=1.0)
                    nc.sync.dma_start(
                        out=outT_dram[:, outT_off + c0:outT_off + c0 + cw],
                        in_=oT[:, :cw])
                    if stats_cols is not None:
                        si, sj = stats_cols
                        part = sbp.tile([P, 1], F32, tag="wst1", name="wst1",
                                        bufs=2)
                        nc.vector.reduce_sum(part[:], oT[:, :cw],
                                             mybir.AxisListType.X)
                        nc.vector.tensor_add(stats_sb[:, si:si + 1],
                                             stats_sb[:, si:si + 1], part[:])
                        trash = sbp.tile([P, WCH], F32, tag="wtrash",
                                         name="wtrash", bufs=2)
                        part2 = sbp.tile([P, 1], F32, tag="wst2", name="wst2",
                                         bufs=2)
                        nc.scalar.activation(
                            trash[:, :cw], oT[:, :cw],
                            mybir.ActivationFunctionType.Square,
                            accum_out=part2[:])
                        nc.vector.tensor_add(stats_sb[:, sj:sj + 1],
                                             stats_sb[:, sj:sj + 1],
                                             part2[:])
                    if rows_dram is not None:
                        _emit_rows(oT, cw, rows_dram, rows_off + c0, sbp, psp)

            def _emit_rows(srcT_sb, cw, rows_dram, row_base, sbp, psp):
                for b0 in range(0, cw, P):
                    bw = min(P, cw - b0)
                    tp = psp.tile([P, P], DT, tag="tps", name="tps", bufs=2)
                    nc.tensor.transpose(tp[:bw, :], srcT_sb[:, b0:b0 + bw],
                                        ident[:])
                    rows = sbp.tile([P, P], DT, tag="rows", name="rows",
                                    bufs=3)
                    nc.scalar.copy(rows[:bw, :], tp[:bw, :])
                    nc.sync.dma_start(
                        out=rows_dram[row_base + b0:row_base + b0 + bw, :],
                        in_=rows[:bw, :])

            # ================= layer 1 =================
            with tc.tile_pool(name="meanA", bufs=1) as mp:
                meanT = mp.tile([P, pa.nsb * pa.sbn], DT, name="meanTA")
                agg_pass(pa, xc, aggA_idx, aggA_dsc, meanT)
                with (
                    tc.tile_pool(name="w1a", bufs=1) as sbp,
                    tc.tile_pool(name="w1ap", bufs=1, space="PSUM") as psp,
                ):
                    w_stage(cfg.apc, meanT, xaT, 0, w_sb["W_msg1_ca"][:],
                            w_sb["W_self1_a"][:], b_sb["b1_a"], True,
                            haT_d, 0, ha_own, 0, None, sbp, psp)
            nc.gpsimd.collective_compute(
                "AllGather", mybir.AluOpType.bypass, replica_groups=rg,
                ins=[ha_own[:]], outs=[ha_full[:]])

            for h in range(2):
                with tc.tile_pool(name=f"meanC{h}", bufs=1) as mp:
                    meanT = mp.tile([P, pcs[h].nsb * pcs[h].sbn], DT,
                                    name="meanTC")
                    agg_pass(pcs[h], xa, aggC_idx[h], aggC_dsc[h], meanT)
                    with (
                        tc.tile_pool(name=f"w1c{h}", bufs=1) as sbp,
                        tc.tile_pool(name=f"w1cp{h}", bufs=1,
                                     space="PSUM") as psp,
                    ):
                        w_stage(cfg.chalf, meanT, xcT, h * cfg.chalf,
                                w_sb["W_msg1_ac"][:], w_sb["W_self1_c"][:],
                                b_sb["b1_c"], True, hcT_d, h * cfg.chalf,
                                hc_own, h * cfg.chalf, None, sbp, psp)
            nc.gpsimd.collective_compute(
                "AllGather", mybir.AluOpType.bypass, replica_groups=rg,
                ins=[hc_own[:]], outs=[hc_full[:]])

            # ================= layer 2 =================
            with tc.tile_pool(name="meanA2", bufs=1) as mp:
                meanT = mp.tile([P, pa.nsb * pa.sbn], DT, name="meanTA2")
                agg_pass(pa, hc_full, aggA_idx, aggA_dsc, meanT)
                with (
                    tc.tile_pool(name="w2a", bufs=1) as sbp,
                    tc.tile_pool(name="w2ap", bufs=1, space="PSUM") as psp,
                ):
                    w_stage(cfg.apc, meanT, haT_d, 0, w_sb["W_msg2_ca"][:],
                            w_sb["W_self2_a"][:], b_sb["b2_a"], False,
                            zaT_d, 0, None, 0, (0, 1), sbp, psp)
            for h in range(2):
                with tc.tile_pool(name=f"meanC2{h}", bufs=1) as mp:
                    meanT = mp.tile([P, pcs[h].nsb * pcs[h].sbn], DT,
                                    name="meanTC2")
                    agg_pass(pcs[h], ha_full, aggC_idx[h], aggC_dsc[h],
                             meanT)
                    with (
                        tc.tile_pool(name=f"w2c{h}", bufs=1) as sbp,
                        tc.tile_pool(name=f"w2cp{h}", bufs=1,
                                     space="PSUM") as psp,
                    ):
                        w_stage(cfg.chalf, meanT, hcT_d, h * cfg.chalf,
                                w_sb["W_msg2_ac"][:], w_sb["W_self2_c"][:],
                                b_sb["b2_c"], False, zcT_d, h * cfg.chalf,
                                None, 0, (2, 3), sbp, psp)

            # ================= BN + U tables =================
            with (
                tc.tile_pool(name="bn", bufs=1) as sbp,
                tc.tile_pool(name="bnp", bufs=1, space="PSUM") as psp,
            ):
                nc.sync.dma_start(out=stats_in[:], in_=stats_sb[:])
                nc.gpsimd.collective_compute(
                    "AllReduce", mybir.AluOpType.add, replica_groups=rg,
                    ins=[stats_in[:]], outs=[stats_out[:]])
                st = sbp.tile([P, 4], F32)
                nc.sync.dma_start(out=st[:], in_=stats_out[:])

                def bn_coeff(si, sj, n, gamma, beta, tagp):
                    mu = sbp.tile([P, 1], F32, name=f"mu{tagp}")
                    nc.vector.tensor_scalar_mul(mu[:], st[:, si:si + 1],
                                                1.0 / n)
                    msq = sbp.tile([P, 1], F32, name=f"msq{tagp}")
                    nc.vector.tensor_scalar_mul(msq[:], st[:, sj:sj + 1],
                                                1.0 / n)
                    mu2 = sbp.tile([P, 1], F32, name=f"mu2{tagp}")
                    nc.vector.tensor_mul(mu2[:], mu[:], mu[:])
                    var = sbp.tile([P, 1], F32, name=f"var{tagp}")
                    nc.vector.tensor_sub(var[:], msq[:], mu2[:])
                    nc.vector.tensor_scalar_add(var[:], var[:], BN_EPS)
                    sd = sbp.tile([P, 1], F32, name=f"sd{tagp}")
                    nc.scalar.activation(sd[:], var[:],
                                         mybir.ActivationFunctionType.Sqrt)
                    rstd = sbp.tile([P, 1], F32, name=f"rstd{tagp}")
                    nc.vector.reciprocal(rstd[:], sd[:])
                    scl = sbp.tile([P, 1], F32, name=f"scl{tagp}")
                    nc.vector.tensor_mul(scl[:], b_sb[gamma][:], rstd[:])
                    mg = sbp.tile([P, 1], F32, name=f"mg{tagp}")
                    nc.vector.tensor_mul(mg[:], mu[:], scl[:])
                    bia = sbp.tile([P, 1], F32, name=f"bia{tagp}")
                    nc.vector.tensor_sub(bia[:], b_sb[beta][:], mg[:])
                    return scl, bia

                scl_a_c, bia_a_c = bn_coeff(0, 1, cfg.n_a, "bn_gamma_a",
                                            "bn_beta_a", "a")
                scl_c_c, bia_c_c = bn_coeff(2, 3, cfg.n_c, "bn_gamma_c",
                                            "bn_beta_c", "c")

                def bn_u(nloc, zT_dram, scl, bia, w1half, ubias, rows_dram):
                    """rows_dram <- rows of bn(z) @ w1half (+ubias)."""
                    for c0 in range(0, nloc, WCH):
                        cw = min(WCH, nloc - c0)
                        zT = sbp.tile([P, WCH], DT, tag="bnz", name="bnz",
                                      bufs=3)
                        nc.sync.dma_start(out=zT[:, :cw],
                                          in_=zT_dram[:, c0:c0 + cw])
                        bnT = sbp.tile([P, WCH], DT, tag="bnt", name="bnt",
                                       bufs=3)
                        nc.scalar.activation(
                            bnT[:, :cw], zT[:, :cw],
                            mybir.ActivationFunctionType.Identity,
                            bias=bia[:], scale=scl[:])
                        ups = psp.tile([P, WCH], F32, tag="ups", name="ups",
                                       bufs=2)
                        nc.tensor.matmul(ups[:, :cw], lhsT=w1half,
                                         rhs=bnT[:, :cw], start=True,
                                         stop=True, skip_group_check=True)
                        uT = sbp.tile([P, WCH], DT, tag="uT", name="uT",
                                      bufs=3)
                        nc.scalar.activation(
                            uT[:, :cw], ups[:, :cw],
                            mybir.ActivationFunctionType.Identity,
                            bias=ubias[:] if ubias is not None else 0.0,
                            scale=1.0)
                        _emit_rows(uT, cw, rows_dram, c0, sbp, psp)

                bn_u(cfg.apc, zaT_d, scl_a_c, bia_a_c, w_sb["Wd1a"][:],
                     None, ua_own)
                bn_u(cfg.cpc, zcT_d, scl_c_c, bia_c_c, w_sb["Wd1c"][:],
                     b_sb["b_dec1"], uc_loc)
            nc.gpsimd.collective_compute(
                "AllGather", mybir.AluOpType.bypass, replica_groups=rg,
                ins=[ua_own[:]], outs=[ua_full[:]])

            # ================= decoder =================
            grp_L = dec["grp_L"]
            goffs = dec["goffs"]
            with tc.tile_pool(name="dec", bufs=1) as sbp:
                ysb = sbp.tile([P, ldT], F32, name="ysb")
                for sub in range(2):
                    for ab in range(cfg.nblk_a):
                        gi = sub * cfg.nblk_a + ab
                        L = int(grp_L[sub, ab])
                        o = int(goffs[gi])
                        uc_rows = min(cfg.zc_sub, cfg.cpc - sub * cfg.zc_sub)
                        ua_rows = min(cfg.srcb_a, cfg.n_a - ab * cfg.srcb_a)
                        for c0 in range(0, L, GCH):
                            cl = min(GCH, L - c0)
                            oc = o + c0
                            ctn = cl // P
                            ixc = sbp.tile([P, GCH // 16], mybir.dt.int16,
                                           tag="dixc", name="dixc", bufs=4)
                            nc.sync.dma_start(
                                out=ixc[:, :cl // 16],
                                in_=dec_idx_c[:, oc // 16:(oc + cl) // 16])
                            ucg = sbp.tile([P, TPC, P], DT, tag="duc",
                                           name="duc", bufs=4)
                            nc.gpsimd.dma_gather(
                                ucg[:, :ctn, :],
                                uc_loc[sub * cfg.zc_sub:
                                       sub * cfg.zc_sub + uc_rows, :],
                                ixc[:, :cl // 16], cl, cl, P,
                                queue_num=next_q())
                            ixa = sbp.tile([P, GCH // 16], mybir.dt.int16,
                                           tag="dixa", name="dixa", bufs=4)
                            nc.sync.dma_start(
                                out=ixa[:, :cl // 16],
                                in_=dec_idx_a[:, oc // 16:(oc + cl) // 16])
                            uag = sbp.tile([P, TPC, P], DT, tag="dua",
                                           name="dua", bufs=4)
                            nc.gpsimd.dma_gather(
                                uag[:, :ctn, :],
                                ua_full[ab * cfg.srcb_a:
                                        ab * cfg.srcb_a + ua_rows, :],
                                ixa[:, :cl // 16], cl, cl, P,
                                queue_num=next_q())
                            ssum = sbp.tile([P, GCH], DT, tag="dsum",
                                            name="dsum", bufs=4)
                            ssum3 = ssum[:, :cl].rearrange(
                                "p (t w) -> p t w", w=P)
                            nc.vector.tensor_tensor(
                                out=ssum3,
                                in0=ucg[:, :ctn, :], in1=uag[:, :ctn, :],
                                op=mybir.AluOpType.add)
                            nc.vector.tensor_scalar(
                                out=ssum[:, :cl], in0=ssum[:, :cl],
                                scalar1=0.0, scalar2=None,
                                op0=mybir.AluOpType.max)
                            nc.vector.tensor_mul(
                                ssum[:, :cl], ssum[:, :cl], w2r_sb[:, :cl])
                            nc.vector.reduce_sum(
                                ysb[:, oc // P:oc // P + ctn],
                                ssum[:, :cl].rearrange(
                                    "p (t w) -> p t w", w=P),
                                mybir.AxisListType.X)
                nc.vector.tensor_scalar(
                    out=ysb[:], in0=ysb[:], scalar1=b_sb["b_dec2c"][:],
                    scalar2=None, op0=mybir.AluOpType.add)
                nc.sync.dma_start(out=y_out[:], in_=ysb[:])

            if dbg:
                nc.sync.dma_start(out=dbg_outs["d_ha"][:], in_=ha_full[:])
                nc.sync.dma_start(out=dbg_outs["d_hc"][:], in_=hc_full[:])
                nc.sync.dma_start(out=dbg_outs["d_ua"][:], in_=ua_full[:])
                nc.sync.dma_start(out=dbg_outs["d_uc"][:], in_=uc_loc[:])
                nc.sync.dma_start(out=dbg_outs["d_st"][:], in_=stats_out[:])

    nc.compile()
    return nc


# ---------------------------------------------------------------------------
# entry point
# ---------------------------------------------------------------------------

def make_in_maps(cfg, inputs, pa, pcs, dec):
    npdt = cfg.npdt
    f = lambda a: np.ascontiguousarray(np.asarray(a), dtype=np.float32)
    xc16 = f(inputs["x_customer"]).astype(npdt)
    xa16 = f(inputs["x_article"]).astype(npdt)
    wd1 = f(inputs["W_dec1"])
    w2 = f(inputs["W_dec2"]).reshape(-1)
    base = dict(
        xc=xc16, xa=xa16,
        W_msg1_ca=f(inputs["W_msg1_ca"]).astype(npdt),
        W_self1_a=f(inputs["W_self1_a"]).astype(npdt),
        W_msg1_ac=f(inputs["W_msg1_ac"]).astype(npdt),
        W_self1_c=f(inputs["W_self1_c"]).astype(npdt),
        W_msg2_ca=f(inputs["W_msg2_ca"]).astype(npdt),
        W_self2_a=f(inputs["W_self2_a"]).astype(npdt),
        W_msg2_ac=f(inputs["W_msg2_ac"]).astype(npdt),
        W_self2_c=f(inputs["W_self2_c"]).astype(npdt),
        Wd1c=wd1[:P].astype(npdt), Wd1a=wd1[P:].astype(npdt),
        w2rep=np.tile(w2.astype(npdt).reshape(1, P), (P, GCH // P)),
        b1_a=f(inputs["b1_a"]).reshape(P, 1),
        b1_c=f(inputs["b1_c"]).reshape(P, 1),
        b2_a=f(inputs["b2_a"]).reshape(P, 1),
        b2_c=f(inputs["b2_c"]).reshape(P, 1),
        bn_gamma_c=f(inputs["bn_gamma_c"]).reshape(P, 1),
        bn_beta_c=f(inputs["bn_beta_c"]).reshape(P, 1),
        bn_gamma_a=f(inputs["bn_gamma_a"]).reshape(P, 1),
        bn_beta_a=f(inputs["bn_beta_a"]).reshape(P, 1),
        b_dec1=f(inputs["b_dec1"]).reshape(P, 1),
        b_dec2c=np.full((P, 1), float(np.asarray(inputs["b_dec2"]).item()),
                        np.float32),
    )
    in_maps = []
    for k in range(NCORES):
        m = dict(base)
        m["xaT"] = np.ascontiguousarray(
            xa16[k * cfg.apc:(k + 1) * cfg.apc].T)
        m["xcT"] = np.ascontiguousarray(
            xc16[k * cfg.cpc:(k + 1) * cfg.cpc].T)
        m["aggA_idx"] = pa.idx[k]
        m["aggA_dsc"] = pa.dsc[k]
        for h in range(2):
            m[f"aggC{h}_idx"] = pcs[h].idx[k]
            m[f"aggC{h}_dsc"] = pcs[h].dsc[k]
        m["dec_idx_c"] = dec["idx_c"][k]
        m["dec_idx_a"] = dec["idx_a"][k]
        in_maps.append(m)
    return in_maps


def run(cfg, inputs, trace=False, dbg=False):
    pa, pcs, dec = prep_all(cfg, inputs)
    in_maps = make_in_maps(cfg, inputs, pa, pcs, dec)
    nc = build_nc(cfg, pa, pcs, dec, dbg=dbg)
    res = run_bass_kernel_spmd(nc, in_maps, core_ids=list(range(NCORES)),
                               trace=trace)
    y = np.empty(cfg.e_lbl, np.float32)
    for k in range(NCORES):
        yl = res.results[k]["y"].T.reshape(-1)
        po = dec["out_pos"][k]
        vm = po >= 0
        y[po[vm]] = yl[vm]
    return y, res


def kernel(**inputs):
    cfg = Cfg()
    y, _ = run(cfg, inputs, trace=False)
    return y



# revision 12
# speedup vs baseline: 1.8055x; 1.8055x over previous
"""Hetero GNN encoder/decoder (SAGE x2 + BN + edge MLP decoder) on 8 trn2 cores.

v2 strategy (vs v1 baseline):
  - 3 gather passes instead of 4: the two C-direction layers share ONE
    512B-per-edge gather from an interleaved [x_a | h_a] table (fused
    L1C+L2C).  Schedule: L1A (gather x_c) -> fused C (gather [x_a|h_a],
    produce h_c AND z_c) -> L2A (gather h_c) -> decoder.
  - mean-scale (1/cnt) folded into a per-dst-column multiply at the
    PSUM->SBUF drain instead of a per-edge multiply in the one-hot:
    halves the DVE work in the aggregation inner loop.
  - gather tables laid out so AllGathers can be split into halves and
    overlapped with compute: customers remapped to
    v = half*150000 + core*18750 + loc%18750, articles to
    v = half*50000 + core*6250 + loc%6250.
  - BN stats AllReduce split per node type (customer stats reduced during
    the L2A pass); U_a table allgathered in halves, decoder loops
    article-block-major so it can start on the first half.

All structure (loop bounds, window emissions) is compile-time and identical
across cores; per-core variation lives in the data (padded to uniform sizes).
"""
import sys

sys.path.insert(0, "/opt/trn_rl_repo")

import numpy as np

import concourse.bacc as bacc
import concourse.bass as bass
import concourse.mybir as mybir
import concourse.tile as tile
from concourse.bass_utils import run_bass_kernel_spmd
from concourse.masks import make_identity

P = 128
NCORES = 8
GCH = 2048          # indices per dma_gather
TPC = GCH // P      # tiles per gather chunk
WCH = 512           # W-stage column chunk
MAXW = 4            # max windows per tile (window-relative encoding)
BN_EPS = 1e-5


class Cfg:
    def __init__(self, n_c=300000, n_a=100000, e_lbl=1000000,
                 sbn=1280, srcb_c=30000, srcb_a=25000):
        self.n_c, self.n_a, self.e_lbl = n_c, n_a, e_lbl
        self.cpc, self.apc = n_c // NCORES, n_a // NCORES
        assert self.cpc * NCORES == n_c and self.apc * NCORES == n_a
        self.chalf = self.cpc // 2      # 18750
        self.ahq = self.apc // 4        # 3125 (xh/ua allgather quarter)
        self.sbn = sbn
        self.sbn_c = 1024
        self.srcb_c, self.srcb_a = srcb_c, srcb_a
        self.nblk_c = n_c // srcb_c     # 10 (exact)
        self.nblk_a = n_a // srcb_a     # 4 (exact)
        assert self.nblk_c * srcb_c == n_c and self.nblk_a * srcb_a == n_a
        assert srcb_c < 32768 and srcb_a < 32768
        self.chg = n_c // 2             # rows per hc_g table (150000)
        self.ahg = n_a // 4             # rows per xh_g table (25000)
        self.zc_sub = self.chalf        # U_c local gather sub-block
        self.dt = mybir.dt.float16
        self.npdt = np.float16


def _ru(x, m):
    return (x + m - 1) // m * m


def _wrap_idx_chunk(flat):
    """[n] int16 -> [128, n/16] wrap (16-partition, replicated x8)."""
    n = flat.shape[0]
    w = flat.astype(np.int16).reshape(n // 16, 16).T
    return np.tile(w, (8, 1))


def _pack_pcol(a):
    """[n] -> [128, n/128]: element i -> partition i%128, col i//128."""
    return np.ascontiguousarray(a.reshape(-1, P).T)


def remap_c(s, cfg):
    """customer id -> position in [hc_g0 | hc_g1] virtual table."""
    q, loc = s // cfg.cpc, s % cfg.cpc
    h, r = loc // cfg.chalf, loc % cfg.chalf
    return h * cfg.chg + q * cfg.chalf + r


def remap_a(g, cfg):
    """article id -> position in xh_g[qt] (= ua_g[qt]) quarter tables."""
    q, loc = g // cfg.apc, g % cfg.apc
    qt, r = loc // cfg.ahq, loc % cfg.ahq
    return qt * cfg.ahg + q * cfg.ahq + r


# ---------------------------------------------------------------------------
# host-side structure + array prep for one aggregation pass
# ---------------------------------------------------------------------------

class AggPass:
    """Static structure (shared across cores) + per-core packed arrays."""

    def __init__(self, name, nloc, srcb, nsrc_blk, sbn):
        self.name = name
        self.nloc = nloc
        self.srcb = srcb
        self.nsrc_blk = nsrc_blk
        self.sbn = sbn
        self.nsb = -(-nloc // sbn)
        self.sb_nodes = [min(sbn, nloc - s * sbn) for s in range(self.nsb)]
        self.run_L = None        # [nsb, nsrc_blk] uniform padded run lengths
        self.etot = 0
        self.emits = None        # per sb: list of (j, t, wlo, nwin) per tile
        self.uncovered = None    # per sb: list of never-touched windows
        self.idx = None          # per core [128, etot/16] int16
        self.dsc = None          # per core [128, etot/128] f16 (dst only)


def prep_agg_pass(name, src, dst_loc, core_e, nloc, srcb, nsrc_blk, sbn):
    """src is already remapped into virtual-table space."""
    ap = AggPass(name, nloc, srcb, nsrc_blk, sbn)
    nsb = ap.nsb
    nruns = nsb * nsrc_blk

    per_core = []
    counts = np.zeros((NCORES, nruns), np.int64)
    for k in range(NCORES):
        m = core_e == k
        s, d = src[m], dst_loc[m]
        j = s // srcb
        sb = d // sbn
        order = np.lexsort((d, j, sb))
        s, d, j, sb = s[order], d[order], j[order], sb[order]
        rid = sb * nsrc_blk + j
        counts[k] = np.bincount(rid, minlength=nruns)
        per_core.append((s, d, rid))

    run_L = _ru(counts.max(axis=0), P)          # uniform, 128-multiple
    offs = np.concatenate([[0], np.cumsum(run_L)]).astype(np.int64)
    etot = int(offs[-1])
    ap.run_L = run_L.reshape(nsb, nsrc_blk)
    ap.etot = etot

    # superblock-relative dst per core (for window structure)
    dstrel_all = np.full((NCORES, etot), -1.0e9, np.float64)
    pos_all = []
    for k in range(NCORES):
        s, d, rid = per_core[k]
        run_start = np.concatenate([[0], np.cumsum(counts[k])])[:-1]
        pos = offs[rid] + (np.arange(len(s)) - run_start[rid])
        pos_all.append(pos)
        dstrel_all[k, pos] = (d - (d // sbn) * sbn).astype(np.float64)

    # per-tile window range (union over cores)
    T = etot // P
    Dw = dstrel_all.reshape(NCORES, T, P)
    valid_any = Dw.max(axis=2) >= 0              # [NCORES, T]
    with np.errstate(invalid="ignore"):
        wlo_c = np.where(Dw >= 0, Dw, np.inf).min(axis=2) // P
        whi_c = np.where(Dw >= 0, Dw, -np.inf).max(axis=2) // P
    wlo_t = np.where(valid_any, wlo_c, np.inf).min(axis=0)
    whi_t = np.where(valid_any, whi_c, -np.inf).max(axis=0)

    emits = []
    uncovered = []
    tile_wlo = np.zeros(T, np.int64)             # per global tile
    for s in range(nsb):
        nwin = -(-ap.sb_nodes[s] // P)
        covered = set()
        sb_emits = []
        for j in range(nsrc_blk):
            r = s * nsrc_blk + j
            o = int(offs[r])
            nt = int(ap.run_L[s, j] // P)
            for t in range(nt):
                g = o // P + t
                if np.isfinite(wlo_t[g]):
                    a = max(0, min(int(wlo_t[g]), nwin - 1))
                    b = max(a, min(int(whi_t[g]), nwin - 1))
                else:
                    a, b = 0, 0
                nw = b - a + 1
                assert nw <= MAXW, f"tile spans {nw} windows"
                covered.update(range(a, b + 1))
                tile_wlo[g] = a
                sb_emits.append((j, t, a, nw))
        emits.append(sb_emits)
        uncovered.append(sorted(set(range(nwin)) - covered))
    ap.emits = emits
    ap.uncovered = uncovered

    # per-core packed arrays (dst window-relative to tile_wlo)
    idxs, dscs = [], []
    for k in range(NCORES):
        s, d, rid = per_core[k]
        pos = pos_all[k]
        idx16 = np.zeros(etot, np.int16)
        idx16[pos] = (s - (s // srcb) * srcb).astype(np.int16)
        dstrel = np.full(etot, -1000.0, np.float32)
        dstrel[pos] = (d - (d // sbn) * sbn).astype(np.float32)
        dstrel -= 128.0 * tile_wlo[np.arange(etot) // P]
        dstrel[dstrel < -1000.0] = -1000.0

        wrapped = np.zeros((P, etot // 16), np.int16)
        dsc = np.zeros((P, etot // P), np.float16)
        run_Lf = run_L.reshape(-1)
        for r in range(nruns):
            o = int(offs[r])
            L = int(run_Lf[r])
            if L == 0:
                continue
            wrapped[:, o // 16:(o + L) // 16] = _wrap_idx_chunk(idx16[o:o + L])
            dsc[:, o // P:(o + L) // P] = \
                _pack_pcol(dstrel[o:o + L]).astype(np.float16)
        idxs.append(wrapped)
        dscs.append(dsc)
    ap.idx, ap.dsc = idxs, dscs
    return ap


# ---------------------------------------------------------------------------
# full host prep
# ---------------------------------------------------------------------------

def prep_all(cfg, inputs):
    i64 = lambda a: np.asarray(a).astype(np.int64)
    e_src = i64(inputs["edge_src_customer"])
    e_dst = i64(inputs["edge_dst_article"])
    l_c = i64(inputs["label_customer"])
    l_a = i64(inputs["label_article"])

    cnt_a = np.bincount(e_dst, minlength=cfg.n_a)
    cnt_c = np.bincount(e_src, minlength=cfg.n_c)
    scl_a = (1.0 / np.maximum(cnt_a, 1.0)).astype(np.float32)
    scl_c = (1.0 / np.maximum(cnt_c, 1.0)).astype(np.float32)

    vc_e = remap_c(e_src, cfg)
    va_e = remap_a(e_dst, cfg)

    # A pass: aggregate customers into articles; src = remapped customer
    pa = prep_agg_pass("A", vc_e, e_dst % cfg.apc, e_dst // cfg.apc,
                       cfg.apc, cfg.srcb_c, cfg.nblk_c, cfg.sbn)
    # C passes (two halves of local customers); src = remapped article
    dloc = e_src % cfg.cpc
    core_c = e_src // cfg.cpc
    half = (dloc >= cfg.chalf).astype(np.int64)
    pcs = []
    for h in range(2):
        m = half == h
        pcs.append(prep_agg_pass(
            f"C{h}", va_e[m], dloc[m] - h * cfg.chalf, core_c[m],
            cfg.chalf, cfg.srcb_a, cfg.nblk_a, cfg.sbn_c))

    # decoder labels: partition by customer core, group by (ablk, sub)
    va_l = remap_a(l_a, cfg)
    core_l = l_c // cfg.cpc
    sub_l = (l_c % cfg.cpc) // cfg.zc_sub
    ablk_l = va_l // cfg.srcb_a
    gid = ablk_l * 2 + sub_l                    # ablk-major
    ngrp = 2 * cfg.nblk_a
    gcounts = np.zeros((NCORES, ngrp), np.int64)
    per_core_lbl = []
    for k in range(NCORES):
        m = core_l == k
        lc, va, g, orig = l_c[m], va_l[m], gid[m], np.nonzero(m)[0]
        order = np.argsort(g, kind="stable")
        lc, va, g, orig = lc[order], va[order], g[order], orig[order]
        gcounts[k] = np.bincount(g, minlength=ngrp)
        per_core_lbl.append((lc, va, g, orig))
    grp_L = _ru(gcounts.max(axis=0), P)
    goffs = np.concatenate([[0], np.cumsum(grp_L)]).astype(np.int64)
    ld_pad = int(goffs[-1])

    dec_idx_c, dec_idx_a, out_pos = [], [], []
    for k in range(NCORES):
        lc, va, g, orig = per_core_lbl[k]
        gstart = np.concatenate([[0], np.cumsum(gcounts[k])])[:-1]
        pos = goffs[g] + (np.arange(len(lc)) - gstart[g])
        ic = np.zeros(ld_pad, np.int16)
        ia = np.zeros(ld_pad, np.int16)
        po = np.full(ld_pad, -1, np.int64)
        lcl = lc % cfg.cpc
        ic[pos] = (lcl - (lcl // cfg.zc_sub) * cfg.zc_sub).astype(np.int16)
        ia[pos] = (va - (va // cfg.srcb_a) * cfg.srcb_a).astype(np.int16)
        po[pos] = orig
        wc = np.zeros((P, ld_pad // 16), np.int16)
        wa = np.zeros((P, ld_pad // 16), np.int16)
        for gi in range(ngrp):
            o = int(goffs[gi])
            L = int(grp_L[gi])
            if L:
                wc[:, o // 16:(o + L) // 16] = _wrap_idx_chunk(ic[o:o + L])
                wa[:, o // 16:(o + L) // 16] = _wrap_idx_chunk(ia[o:o + L])
        dec_idx_c.append(wc)
        dec_idx_a.append(wa)
        out_pos.append(po)

    dec = dict(grp_L=grp_L.reshape(cfg.nblk_a, 2), goffs=goffs, ld_pad=ld_pad,
               idx_c=dec_idx_c, idx_a=dec_idx_a, out_pos=out_pos)
    return pa, pcs, dec, scl_a, scl_c


# ---------------------------------------------------------------------------
# kernel builder
# ---------------------------------------------------------------------------

F32 = mybir.dt.float32


def build_nc(cfg, pa, pcs, dec):
    DT = cfg.dt
    nc = bacc.Bacc("TRN2", target_bir_lowering=False, debug=False,
                   num_devices=NCORES, num_swdge_queues=4)
    qctr = [0]
    def next_q():
        qctr[0] = (qctr[0] + 1) % 4
        return qctr[0]

    ei = lambda n, s, d: nc.dram_tensor(n, s, d, kind="ExternalInput")
    xc_remap = ei("xc_remap", [cfg.n_c, P], DT)
    xa_rows = ei("xa_rows", [cfg.apc, P], DT)
    xaT = ei("xaT", [P, cfg.apc], DT)
    xcT = ei("xcT", [P, cfg.cpc], DT)
    aggA_idx = ei("aggA_idx", [P, pa.etot // 16], mybir.dt.int16)
    aggA_dsc = ei("aggA_dsc", [P, pa.etot // P], DT)
    aggC_idx = [ei(f"aggC{h}_idx", [P, pcs[h].etot // 16], mybir.dt.int16)
                for h in range(2)]
    aggC_dsc = [ei(f"aggC{h}_dsc", [P, pcs[h].etot // P], DT)
                for h in range(2)]
    scl_a_in = ei("scl_a", [P, cfg.apc], F32)
    scl_c_in = ei("scl_c", [P, cfg.cpc], F32)
    dec_idx_c = ei("dec_idx_c", [P, dec["ld_pad"] // 16], mybir.dt.int16)
    dec_idx_a = ei("dec_idx_a", [P, dec["ld_pad"] // 16], mybir.dt.int16)

    wnames = ["W_msg1_ca", "W_self1_a", "W_msg1_ac", "W_self1_c",
              "W_msg2_ca", "W_self2_a", "W_msg2_ac", "W_self2_c",
              "Wd1c", "Wd1a"]
    wts = {n: ei(n, [P, P], DT) for n in wnames}
    w2rep = ei("w2rep", [P, GCH], DT)       # W_dec2 repeated per 128-segment
    bnames = ["b1_a", "b1_c", "b2_a", "b2_c",
              "bn_gamma_c", "bn_beta_c", "bn_gamma_a", "bn_beta_a",
              "b_dec1", "b_dec2c"]
    bis = {n: ei(n, [P, 1], F32) for n in bnames}

    ldT = dec["ld_pad"] // P
    y_out = nc.dram_tensor("y", [P, ldT], F32, kind="ExternalOutput")

    rg = [list(range(NCORES))]

    with tile.TileContext(nc) as tc:
        with (
            tc.tile_pool(name="dramp", bufs=1, space="DRAM") as dramp,
            tc.tile_pool(name="const", bufs=1) as cs,
        ):
            xh_own = dramp.tile([cfg.apc, 2 * P], DT)
            xh_g = [dramp.tile([cfg.ahg, 2 * P], DT, addr_space="Shared",
                               name=f"xh_g{i}") for i in range(2)]
            hc_own = dramp.tile([cfg.cpc, P], DT)
            hc_g = [dramp.tile([cfg.chg, P], DT, addr_space="Shared",
                               name=f"hc_g{i}") for i in range(2)]
            ua_own = dramp.tile([cfg.apc, P], DT)
            ua_g = [dramp.tile([cfg.ahg, P], DT, addr_space="Shared",
                               name=f"ua_g{i}") for i in range(2)]
            uc_loc = dramp.tile([cfg.cpc, P], DT)
            haT_d = dramp.tile([P, cfg.apc], DT)
            zaT_d = dramp.tile([P, cfg.apc], DT)
            zcT_d = dramp.tile([P, cfg.cpc], DT)
            stats_c_in = dramp.tile([P, 2], F32)
            stats_c_out = dramp.tile([P, 2], F32, addr_space="Shared")
            stats_a_in = dramp.tile([P, 2], F32)
            stats_a_out = dramp.tile([P, 2], F32, addr_space="Shared")

            # constants: batched iota tiles (values wi*128 + col%128)
            iota8 = []
            for wi in range(MAXW):
                ii = cs.tile([P, GCH], mybir.dt.int32, name=f"ioi{wi}")
                nc.gpsimd.iota(ii[:], pattern=[[0, TPC], [1, P]],
                               base=wi * P, channel_multiplier=0)
                ff = cs.tile([P, GCH], DT, name=f"iof{wi}")
                nc.vector.tensor_copy(ff[:], ii[:])
                iota8.append(ff)
            ident = cs.tile([P, P], DT)
            make_identity(nc, ident[:])
            w_sb = {n: cs.tile([P, P], DT, name=f"w_{n}") for n in wnames}
            for n in wnames:
                nc.sync.dma_start(out=w_sb[n][:], in_=wts[n][:])
            w2r_sb = cs.tile([P, GCH], DT)
            nc.sync.dma_start(out=w2r_sb[:], in_=w2rep[:])
            b_sb = {n: cs.tile([P, 1], F32, name=f"b_{n}") for n in bnames}
            for n in bnames:
                nc.sync.dma_start(out=b_sb[n][:], in_=bis[n][:])
            zeros_sb = cs.tile([P, GCH], DT)
            nc.vector.memset(zeros_sb[:], 0.0)
            stats_c_sb = cs.tile([P, 2], F32)
            nc.vector.memset(stats_c_sb[:], 0.0)
            stats_a_sb = cs.tile([P, 2], F32)
            nc.vector.memset(stats_a_sb[:], 0.0)

            # stage x_a rows into the interleaved xh table
            nc.sync.dma_start(out=xh_own[:, 0:P], in_=xa_rows[:])

            # ---------------- aggregation pass ----------------
            def agg_pass(ps, table_fn, nfeat, idx_d, dsc_d, drains):
                """drains: list of (meanT_sb, scl_dram, scl_off) per 128-feat
                slice of the gathered rows."""
                nstream = len(drains)
                offs = np.concatenate(
                    [[0], np.cumsum(ps.run_L.reshape(-1))]).astype(np.int64)
                with (
                    tc.tile_pool(name=f"ag_{ps.name}", bufs=1) as sbp,
                    tc.tile_pool(name=f"agp_{ps.name}", bufs=2,
                                 space="PSUM") as psp,
                ):
                    psum_bufs = 2 if nstream == 1 else 1
                    for s in range(ps.nsb):
                        nodes = ps.sb_nodes[s]
                        nwin = -(-nodes // P)
                        psums = [psp.tile([P, nwin * P], F32,
                                          tag=f"aggps{si}",
                                          name=f"psum_agg{si}",
                                          bufs=psum_bufs)
                                 for si in range(nstream)]
                        # first/last touch per PSUM bank (4 windows/bank)
                        touch = {}
                        for (j, t, wlo, nw) in ps.emits[s]:
                            for wi in range(nw):
                                w = wlo + wi
                                touch.setdefault(w // 4, []).append((j, t, w))
                        firsts = {b: v[0] for b, v in touch.items()}
                        lasts = {b: v[-1] for b, v in touch.items()}

                        by_run = {}
                        for e in ps.emits[s]:
                            by_run.setdefault(e[0], []).append(e)
                        for j in sorted(by_run):
                            r = s * ps.nsrc_blk + j
                            o = int(offs[r])
                            L = int(ps.run_L[s, j])
                            nt = L // P
                            idx_sb = sbp.tile([P, L // 16], mybir.dt.int16,
                                              tag="gidx", name="gidx",
                                              bufs=3)
                            nc.sync.dma_start(
                                out=idx_sb[:],
                                in_=idx_d[:, o // 16:(o + L) // 16])
                            dsc_sb = sbp.tile([P, nt], DT, tag="gdsc",
                                              name="gdsc", bufs=3)
                            nc.sync.dma_start(
                                out=dsc_sb[:],
                                in_=dsc_d[:, o // P:o // P + nt])
                            x_tiles = []
                            for c0 in range(0, L, GCH):
                                cl = min(GCH, L - c0)
                                x = sbp.tile([P, TPC, nfeat], DT, tag="gx",
                                             name="gx",
                                             bufs=6 if nfeat == P else 4)
                                nc.gpsimd.dma_gather(
                                    x[:, :cl // P, :], table_fn(j),
                                    idx_sb[:, c0 // 16:(c0 + cl) // 16],
                                    cl, cl, nfeat, queue_num=next_q())
                                x_tiles.append(x)
                            # per gather-chunk batched one-hot builds
                            run_emits = by_run[j]
                            p8s = {}     # (chunk, wi) -> tile
                            for c in range(0, nt, TPC):
                                ctn = min(TPC, nt - c)
                                maxnw = max(e[3] for e in run_emits
                                            if c <= e[1] < c + ctn)
                                dstb = dsc_sb[:, c:c + ctn] \
                                    .to_broadcast([P, ctn, P])
                                for wi in range(maxnw):
                                    p8 = sbp.tile([P, GCH], DT, tag="gp",
                                                  name="gp", bufs=6)
                                    p83 = p8[:, :ctn * P].rearrange(
                                        "p (t w) -> p t w", w=P)
                                    nc.vector.tensor_tensor(
                                        out=p83,
                                        in0=iota8[wi][:, :ctn * P].rearrange(
                                            "p (t w) -> p t w", w=P),
                                        in1=dstb,
                                        op=mybir.AluOpType.is_equal)
                                    p8s[(c // TPC, wi)] = p8
                            for (j2, t, wlo, nw) in run_emits:
                                for wi in range(nw):
                                    w = wlo + wi
                                    p8 = p8s[(t // TPC, wi)]
                                    for si in range(nstream):
                                        nc.tensor.matmul(
                                            psums[si][:, w * P:(w + 1) * P],
                                            lhsT=x_tiles[t // TPC]
                                                [:, t % TPC,
                                                 si * P:(si + 1) * P],
                                            rhs=p8[:, (t % TPC) * P:
                                                   (t % TPC + 1) * P],
                                            start=(firsts[w // 4]
                                                   == (j2, t, w)),
                                            stop=(lasts[w // 4]
                                                  == (j2, t, w)),
                                            skip_group_check=True)
                        sclt = sbp.tile([P, ps.sbn], F32, tag="sclt",
                                        name="sclt", bufs=2)
                        col = drains[0][2] + s * ps.sbn
                        nc.sync.dma_start(
                            out=sclt[:, :nodes],
                            in_=drains[0][1][:, col:col + nodes])
                        for si, (meanT_sb, scl_dram, scl_off) in \
                                enumerate(drains):
                            nc.vector.tensor_tensor(
                                out=meanT_sb[:, s * ps.sbn:
                                             s * ps.sbn + nodes],
                                in0=psums[si][:, :nodes],
                                in1=sclt[:, :nodes],
                                op=mybir.AluOpType.mult)
                            for w in ps.uncovered[s]:
                                a = s * ps.sbn + w * P
                                b = min(a + P, s * ps.sbn + nodes)
                                nc.vector.memset(meanT_sb[:, a:b], 0.0)

            def _emit_rows(srcT_sb, cw, rows_ap_fn, row_base, sbp, psp):
                for b0 in range(0, cw, P):
                    bw = min(P, cw - b0)
                    tp = psp.tile([P, P], DT, tag="tps", name="tps", bufs=2)
                    nc.tensor.transpose(tp[:bw, :], srcT_sb[:, b0:b0 + bw],
                                        ident[:])
                    rows = sbp.tile([P, P], DT, tag="rows", name="rows",
                                    bufs=3)
                    nc.scalar.copy(rows[:bw, :], tp[:bw, :])
                    nc.sync.dma_start(
                        out=rows_ap_fn(row_base + b0, bw),
                        in_=rows[:bw, :])

            # ================= L1A =================
            with tc.tile_pool(name="meanA", bufs=1) as mp:
                meanT = mp.tile([P, pa.nsb * pa.sbn], DT, name="meanTA")
                agg_pass(pa, lambda j: xc_remap[j * cfg.srcb_c:
                                                (j + 1) * cfg.srcb_c, :],
                         P, aggA_idx, aggA_dsc, [(meanT, scl_a_in, 0)])
                with (
                    tc.tile_pool(name="w1a", bufs=1) as sbp,
                    tc.tile_pool(name="w1ap", bufs=1, space="PSUM") as psp,
                ):
                    for hh in range(2):
                        lo, hi = hh * cfg.ahalf, (hh + 1) * cfg.ahalf
                        for c0 in range(lo, hi, WCH):
                            cw = min(WCH, hi - c0)
                            sT = sbp.tile([P, WCH], DT, tag="wself",
                                          name="wselfT", bufs=3)
                            nc.sync.dma_start(out=sT[:, :cw],
                                              in_=xaT[:, c0:c0 + cw])
                            psum = psp.tile([P, WCH], F32, tag="wps",
                                            name="wps", bufs=3)
                            nc.tensor.matmul(psum[:, :cw],
                                             lhsT=w_sb["W_msg1_ca"][:],
                                             rhs=meanT[:, c0:c0 + cw],
                                             start=True, stop=False,
                                             skip_group_check=True)
                            nc.tensor.matmul(psum[:, :cw],
                                             lhsT=w_sb["W_self1_a"][:],
                                             rhs=sT[:, :cw],
                                             start=False, stop=True,
                                             skip_group_check=True)
                            oT = sbp.tile([P, WCH], DT, tag="woT",
                                          name="woT", bufs=3)
                            nc.scalar.activation(
                                oT[:, :cw], psum[:, :cw],
                                mybir.ActivationFunctionType.Relu,
                                bias=b_sb["b1_a"][:], scale=1.0)
                            nc.sync.dma_start(
                                out=haT_d[:, c0:c0 + cw], in_=oT[:, :cw])
                            _emit_rows(
                                oT, cw,
                                lambda r0, bw: xh_own[r0:r0 + bw, P:2 * P],
                                c0, sbp, psp)
                        nc.gpsimd.collective_compute(
                            "AllGather", mybir.AluOpType.bypass,
                            replica_groups=rg,
                            ins=[xh_own[lo:hi, :]], outs=[xh_g[hh][:]])

            # ================= fused C (L1C + L2C) =================
            def xh_table(j):
                g = j // 2
                jj = j % 2
                return xh_g[g][jj * cfg.srcb_a:(jj + 1) * cfg.srcb_a, :]

            for h in range(2):
                coff = h * cfg.chalf
                with tc.tile_pool(name=f"meanC{h}", bufs=1) as mp:
                    mean1 = mp.tile([P, pcs[h].nsb * pcs[h].sbn], DT,
                                    name="meanTC1")
                    mean2 = mp.tile([P, pcs[h].nsb * pcs[h].sbn], DT,
                                    name="meanTC2")
                    agg_pass(pcs[h], xh_table, 2 * P,
                             aggC_idx[h], aggC_dsc[h],
                             [(mean1, scl_c_in, coff),
                              (mean2, scl_c_in, coff)])
                    with (
                        tc.tile_pool(name=f"w1c{h}", bufs=1) as sbp,
                        tc.tile_pool(name=f"w1cp{h}", bufs=1,
                                     space="PSUM") as psp,
                    ):
                        for c0 in range(0, cfg.chalf, WCH):
                            cw = min(WCH, cfg.chalf - c0)
                            sT = sbp.tile([P, WCH], DT, tag="wself",
                                          name="wselfT", bufs=3)
                            nc.sync.dma_start(
                                out=sT[:, :cw],
                                in_=xcT[:, coff + c0:coff + c0 + cw])
                            ps1 = psp.tile([P, WCH], F32, tag="wps1",
                                           name="wps1", bufs=2)
                            nc.tensor.matmul(ps1[:, :cw],
                                             lhsT=w_sb["W_msg1_ac"][:],
                                             rhs=mean1[:, c0:c0 + cw],
                                             start=True, stop=False,
                                             skip_group_check=True)
                            nc.tensor.matmul(ps1[:, :cw],
                                             lhsT=w_sb["W_self1_c"][:],
                                             rhs=sT[:, :cw],
                                             start=False, stop=True,
                                             skip_group_check=True)
                            hT = sbp.tile([P, WCH], DT, tag="whT",
                                          name="whT", bufs=3)
                            nc.scalar.activation(
                                hT[:, :cw], ps1[:, :cw],
                                mybir.ActivationFunctionType.Relu,
                                bias=b_sb["b1_c"][:], scale=1.0)
                            _emit_rows(
                                hT, cw,
                                lambda r0, bw: hc_own[r0:r0 + bw, :],
                                coff + c0, sbp, psp)
                            ps2 = psp.tile([P, WCH], F32, tag="wps2",
                                           name="wps2", bufs=2)
                            nc.tensor.matmul(ps2[:, :cw],
                                             lhsT=w_sb["W_msg2_ac"][:],
                                             rhs=mean2[:, c0:c0 + cw],
                                             start=True, stop=False,
                                             skip_group_check=True)
                            nc.tensor.matmul(ps2[:, :cw],
                                             lhsT=w_sb["W_self2_c"][:],
                                             rhs=hT[:, :cw],
                                             start=False, stop=True,
                                             skip_group_check=True)
                            zT = sbp.tile([P, WCH], DT, tag="wzT",
                                          name="wzT", bufs=3)
                            nc.scalar.activation(
                                zT[:, :cw], ps2[:, :cw],
                                mybir.ActivationFunctionType.Identity,
                                bias=b_sb["b2_c"][:], scale=1.0)
                            nc.sync.dma_start(
                                out=zcT_d[:, coff + c0:coff + c0 + cw],
                                in_=zT[:, :cw])
                            part = sbp.tile([P, 1], F32, tag="wst1",
                                            name="wst1", bufs=2)
                            nc.vector.reduce_sum(part[:], zT[:, :cw],
                                                 mybir.AxisListType.X)
                            nc.vector.tensor_add(stats_c_sb[:, 0:1],
                                                 stats_c_sb[:, 0:1],
                                                 part[:])
                            trash = sbp.tile([P, WCH], F32, tag="wtrash",
                                             name="wtrash", bufs=2)
                            part2 = sbp.tile([P, 1], F32, tag="wst2",
                                             name="wst2", bufs=2)
                            nc.scalar.activation(
                                trash[:, :cw], zT[:, :cw],
                                mybir.ActivationFunctionType.Square,
                                accum_out=part2[:])
                            nc.vector.tensor_add(stats_c_sb[:, 1:2],
                                                 stats_c_sb[:, 1:2],
                                                 part2[:])
            nc.gpsimd.collective_compute(
                "AllGather", mybir.AluOpType.bypass, replica_groups=rg,
                ins=[hc_own[0:cfg.chalf, :]], outs=[hc_g[0][:]])
            nc.gpsimd.collective_compute(
                "AllGather", mybir.AluOpType.bypass, replica_groups=rg,
                ins=[hc_own[cfg.chalf:, :]], outs=[hc_g[1][:]])
            nc.sync.dma_start(out=stats_c_in[:], in_=stats_c_sb[:])
            nc.gpsimd.collective_compute(
                "AllReduce", mybir.AluOpType.add, replica_groups=rg,
                ins=[stats_c_in[:]], outs=[stats_c_out[:]])

            # ================= L2A =================
            def hc_table(j):
                g = j // 5
                jj = j % 5
                return hc_g[g][jj * cfg.srcb_c:(jj + 1) * cfg.srcb_c, :]

            with tc.tile_pool(name="meanA2", bufs=1) as mp:
                meanT = mp.tile([P, pa.nsb * pa.sbn], DT, name="meanTA2")
                agg_pass(pa, hc_table, P, aggA_idx, aggA_dsc,
                         [(meanT, scl_a_in, 0)])
                with (
                    tc.tile_pool(name="w2a", bufs=1) as sbp,
                    tc.tile_pool(name="w2ap", bufs=1, space="PSUM") as psp,
                ):
                    for c0 in range(0, cfg.apc, WCH):
                        cw = min(WCH, cfg.apc - c0)
                        sT = sbp.tile([P, WCH], DT, tag="wself",
                                      name="wselfT", bufs=3)
                        nc.sync.dma_start(out=sT[:, :cw],
                                          in_=haT_d[:, c0:c0 + cw])
                        psum = psp.tile([P, WCH], F32, tag="wps",
                                        name="wps", bufs=3)
                        nc.tensor.matmul(psum[:, :cw],
                                         lhsT=w_sb["W_msg2_ca"][:],
                                         rhs=meanT[:, c0:c0 + cw],
                                         start=True, stop=False,
                                         skip_group_check=True)
                        nc.tensor.matmul(psum[:, :cw],
                                         lhsT=w_sb["W_self2_a"][:],
                                         rhs=sT[:, :cw],
                                         start=False, stop=True,
                                         skip_group_check=True)
                        zT = sbp.tile([P, WCH], DT, tag="woT",
                                      name="woT", bufs=3)
                        nc.scalar.activation(
                            zT[:, :cw], psum[:, :cw],
                            mybir.ActivationFunctionType.Identity,
                            bias=b_sb["b2_a"][:], scale=1.0)
                        nc.sync.dma_start(out=zaT_d[:, c0:c0 + cw],
                                          in_=zT[:, :cw])
                        part = sbp.tile([P, 1], F32, tag="wst1",
                                        name="wst1", bufs=2)
                        nc.vector.reduce_sum(part[:], zT[:, :cw],
                                             mybir.AxisListType.X)
                        nc.vector.tensor_add(stats_a_sb[:, 0:1],
                                             stats_a_sb[:, 0:1], part[:])
                        trash = sbp.tile([P, WCH], F32, tag="wtrash",
                                         name="wtrash", bufs=2)
                        part2 = sbp.tile([P, 1], F32, tag="wst2",
                                         name="wst2", bufs=2)
                        nc.scalar.activation(
                            trash[:, :cw], zT[:, :cw],
                            mybir.ActivationFunctionType.Square,
                            accum_out=part2[:])
                        nc.vector.tensor_add(stats_a_sb[:, 1:2],
                                             stats_a_sb[:, 1:2], part2[:])
            nc.sync.dma_start(out=stats_a_in[:], in_=stats_a_sb[:])
            nc.gpsimd.collective_compute(
                "AllReduce", mybir.AluOpType.add, replica_groups=rg,
                ins=[stats_a_in[:]], outs=[stats_a_out[:]])

            # ================= BN + U tables =================
            with (
                tc.tile_pool(name="bn", bufs=1) as sbp,
                tc.tile_pool(name="bnp", bufs=1, space="PSUM") as psp,
            ):
                def bn_coeff(stats_out_d, n, gamma, beta, tagp):
                    st = sbp.tile([P, 2], F32, name=f"st{tagp}")
                    nc.sync.dma_start(out=st[:], in_=stats_out_d[:])
                    mu = sbp.tile([P, 1], F32, name=f"mu{tagp}")
                    nc.vector.tensor_scalar_mul(mu[:], st[:, 0:1], 1.0 / n)
                    msq = sbp.tile([P, 1], F32, name=f"msq{tagp}")
                    nc.vector.tensor_scalar_mul(msq[:], st[:, 1:2], 1.0 / n)
                    mu2 = sbp.tile([P, 1], F32, name=f"mu2{tagp}")
                    nc.vector.tensor_mul(mu2[:], mu[:], mu[:])
                    var = sbp.tile([P, 1], F32, name=f"var{tagp}")
                    nc.vector.tensor_sub(var[:], msq[:], mu2[:])
                    nc.vector.tensor_scalar_add(var[:], var[:], BN_EPS)
                    sd = sbp.tile([P, 1], F32, name=f"sd{tagp}")
                    nc.scalar.activation(sd[:], var[:],
                                         mybir.ActivationFunctionType.Sqrt)
                    rstd = sbp.tile([P, 1], F32, name=f"rstd{tagp}")
                    nc.vector.reciprocal(rstd[:], sd[:])
                    scl = sbp.tile([P, 1], F32, name=f"scl{tagp}")
                    nc.vector.tensor_mul(scl[:], b_sb[gamma][:], rstd[:])
                    mg = sbp.tile([P, 1], F32, name=f"mg{tagp}")
                    nc.vector.tensor_mul(mg[:], mu[:], scl[:])
                    bia = sbp.tile([P, 1], F32, name=f"bia{tagp}")
                    nc.vector.tensor_sub(bia[:], b_sb[beta][:], mg[:])
                    return scl, bia

                def bn_u(lo, hi, zT_dram, scl, bia, w1half, ubias,
                         rows_dram):
                    for c0 in range(lo, hi, WCH):
                        cw = min(WCH, hi - c0)
                        zT = sbp.tile([P, WCH], DT, tag="bnz", name="bnz",
                                      bufs=3)
                        nc.sync.dma_start(out=zT[:, :cw],
                                          in_=zT_dram[:, c0:c0 + cw])
                        bnT = sbp.tile([P, WCH], DT, tag="bnt", name="bnt",
                                       bufs=3)
                        nc.scalar.activation(
                            bnT[:, :cw], zT[:, :cw],
                            mybir.ActivationFunctionType.Identity,
                            bias=bia[:], scale=scl[:])
                        ups = psp.tile([P, WCH], F32, tag="ups", name="ups",
                                       bufs=2)
                        nc.tensor.matmul(ups[:, :cw], lhsT=w1half,
                                         rhs=bnT[:, :cw], start=True,
                                         stop=True, skip_group_check=True)
                        uT = sbp.tile([P, WCH], DT, tag="uT", name="uT",
                                      bufs=3)
                        nc.scalar.activation(
                            uT[:, :cw], ups[:, :cw],
                            mybir.ActivationFunctionType.Identity,
                            bias=ubias[:] if ubias is not None else 0.0,
                            scale=1.0)
                        _emit_rows(
                            uT, cw,
                            lambda r0, bw: rows_dram[r0:r0 + bw, :],
                            c0, sbp, psp)

                # customer side first: stats_c allreduce landed during L2A
                scl_c_c, bia_c_c = bn_coeff(stats_c_out, cfg.n_c,
                                            "bn_gamma_c", "bn_beta_c", "c")
                bn_u(0, cfg.cpc, zcT_d, scl_c_c, bia_c_c, w_sb["Wd1c"][:],
                     b_sb["b_dec1"], uc_loc)
                scl_a_c, bia_a_c = bn_coeff(stats_a_out, cfg.n_a,
                                            "bn_gamma_a", "bn_beta_a", "a")
                for hh in range(2):
                    lo, hi = hh * cfg.ahalf, (hh + 1) * cfg.ahalf
                    bn_u(lo, hi, zaT_d, scl_a_c, bia_a_c, w_sb["Wd1a"][:],
                         None, ua_own)
                    nc.gpsimd.collective_compute(
                        "AllGather", mybir.AluOpType.bypass,
                        replica_groups=rg,
                        ins=[ua_own[lo:hi, :]], outs=[ua_g[hh][:]])

            # ================= decoder =================
            grp_L = dec["grp_L"]
            goffs = dec["goffs"]
            with tc.tile_pool(name="dec", bufs=1) as sbp:
                ysb = sbp.tile([P, ldT], F32, name="ysb")
                for ab in range(cfg.nblk_a):
                    ua_tab = ua_g[ab // 2][(ab % 2) * cfg.srcb_a:
                                           (ab % 2 + 1) * cfg.srcb_a, :]
                    for sub in range(2):
                        gi = ab * 2 + sub
                        L = int(grp_L[ab, sub])
                        o = int(goffs[gi])
                        for c0 in range(0, L, GCH):
                            cl = min(GCH, L - c0)
                            oc = o + c0
                            ctn = cl // P
                            ixc = sbp.tile([P, GCH // 16], mybir.dt.int16,
                                           tag="dixc", name="dixc", bufs=6)
                            nc.sync.dma_start(
                                out=ixc[:, :cl // 16],
                                in_=dec_idx_c[:, oc // 16:(oc + cl) // 16])
                            ucg = sbp.tile([P, TPC, P], DT, tag="duc",
                                           name="duc", bufs=4)
                            nc.gpsimd.dma_gather(
                                ucg[:, :ctn, :],
                                uc_loc[sub * cfg.zc_sub:
                                       (sub + 1) * cfg.zc_sub, :],
                                ixc[:, :cl // 16], cl, cl, P,
                                queue_num=next_q())
                            ixa = sbp.tile([P, GCH // 16], mybir.dt.int16,
                                           tag="dixa", name="dixa", bufs=6)
                            nc.sync.dma_start(
                                out=ixa[:, :cl // 16],
                                in_=dec_idx_a[:, oc // 16:(oc + cl) // 16])
                            uag = sbp.tile([P, TPC, P], DT, tag="dua",
                                           name="dua", bufs=4)
                            nc.gpsimd.dma_gather(
                                uag[:, :ctn, :], ua_tab,
                                ixa[:, :cl // 16], cl, cl, P,
                                queue_num=next_q())
                            ssum = sbp.tile([P, GCH], DT, tag="dsum",
                                            name="dsum", bufs=6)
                            ssum3 = ssum[:, :cl].rearrange(
                                "p (t w) -> p t w", w=P)
                            nc.vector.tensor_tensor(
                                out=ssum3,
                                in0=ucg[:, :ctn, :], in1=uag[:, :ctn, :],
                                op=mybir.AluOpType.add)
                            nc.vector.tensor_tensor(
                                out=ssum[:, :cl], in0=ssum[:, :cl],
                                in1=zeros_sb[:, :cl],
                                op=mybir.AluOpType.max)
                            nc.vector.tensor_mul(
                                ssum[:, :cl], ssum[:, :cl], w2r_sb[:, :cl])
                            nc.vector.reduce_sum(
                                ysb[:, oc // P:oc // P + ctn],
                                ssum[:, :cl].rearrange(
                                    "p (t w) -> p t w", w=P),
                                mybir.AxisListType.X)
                nc.vector.tensor_scalar(
                    out=ysb[:], in0=ysb[:], scalar1=b_sb["b_dec2c"][:],
                    scalar2=None, op0=mybir.AluOpType.add)
                nc.sync.dma_start(out=y_out[:], in_=ysb[:])

    nc.compile()
    return nc


# ---------------------------------------------------------------------------
# entry point
# ---------------------------------------------------------------------------

def make_in_maps(cfg, inputs, pa, pcs, dec, scl_a, scl_c):
    npdt = cfg.npdt
    f = lambda a: np.ascontiguousarray(np.asarray(a), dtype=np.float32)
    xc16 = f(inputs["x_customer"]).astype(npdt)
    xa16 = f(inputs["x_article"]).astype(npdt)
    wd1 = f(inputs["W_dec1"])
    w2 = f(inputs["W_dec2"]).reshape(-1)

    vc = remap_c(np.arange(cfg.n_c), cfg)
    xc_remap = np.empty_like(xc16)
    xc_remap[vc] = xc16

    base = dict(
        xc_remap=xc_remap,
        W_msg1_ca=f(inputs["W_msg1_ca"]).astype(npdt),
        W_self1_a=f(inputs["W_self1_a"]).astype(npdt),
        W_msg1_ac=f(inputs["W_msg1_ac"]).astype(npdt),
        W_self1_c=f(inputs["W_self1_c"]).astype(npdt),
        W_msg2_ca=f(inputs["W_msg2_ca"]).astype(npdt),
        W_self2_a=f(inputs["W_self2_a"]).astype(npdt),
        W_msg2_ac=f(inputs["W_msg2_ac"]).astype(npdt),
        W_self2_c=f(inputs["W_self2_c"]).astype(npdt),
        Wd1c=wd1[:P].astype(npdt), Wd1a=wd1[P:].astype(npdt),
        w2rep=np.tile(w2.astype(npdt).reshape(1, P), (P, GCH // P)),
        b1_a=f(inputs["b1_a"]).reshape(P, 1),
        b1_c=f(inputs["b1_c"]).reshape(P, 1),
        b2_a=f(inputs["b2_a"]).reshape(P, 1),
        b2_c=f(inputs["b2_c"]).reshape(P, 1),
        bn_gamma_c=f(inputs["bn_gamma_c"]).reshape(P, 1),
        bn_beta_c=f(inputs["bn_beta_c"]).reshape(P, 1),
        bn_gamma_a=f(inputs["bn_gamma_a"]).reshape(P, 1),
        bn_beta_a=f(inputs["bn_beta_a"]).reshape(P, 1),
        b_dec1=f(inputs["b_dec1"]).reshape(P, 1),
        b_dec2c=np.full((P, 1), float(np.asarray(inputs["b_dec2"]).item()),
                        np.float32),
    )
    in_maps = []
    for k in range(NCORES):
        m = dict(base)
        m["xa_rows"] = np.ascontiguousarray(
            xa16[k * cfg.apc:(k + 1) * cfg.apc])
        m["xaT"] = np.ascontiguousarray(
            xa16[k * cfg.apc:(k + 1) * cfg.apc].T)
        m["xcT"] = np.ascontiguousarray(
            xc16[k * cfg.cpc:(k + 1) * cfg.cpc].T)
        m["scl_a"] = np.ascontiguousarray(np.broadcast_to(
            scl_a[k * cfg.apc:(k + 1) * cfg.apc].reshape(1, -1),
            (P, cfg.apc)))
        m["scl_c"] = np.ascontiguousarray(np.broadcast_to(
            scl_c[k * cfg.cpc:(k + 1) * cfg.cpc].reshape(1, -1),
            (P, cfg.cpc)))
        m["aggA_idx"] = pa.idx[k]
        m["aggA_dsc"] = pa.dsc[k]
        for h in range(2):
            m[f"aggC{h}_idx"] = pcs[h].idx[k]
            m[f"aggC{h}_dsc"] = pcs[h].dsc[k]
        m["dec_idx_c"] = dec["idx_c"][k]
        m["dec_idx_a"] = dec["idx_a"][k]
        in_maps.append(m)
    return in_maps


def run(cfg, inputs, trace=False):
    pa, pcs, dec, scl_a, scl_c = prep_all(cfg, inputs)
    in_maps = make_in_maps(cfg, inputs, pa, pcs, dec, scl_a, scl_c)
    nc = build_nc(cfg, pa, pcs, dec)
    res = run_bass_kernel_spmd(nc, in_maps, core_ids=list(range(NCORES)),
                               trace=trace)
    y = np.empty(cfg.e_lbl, np.float32)
    for k in range(NCORES):
        yl = res.results[k]["y"].T.reshape(-1)
        po = dec["out_pos"][k]
        vm = po >= 0
        y[po[vm]] = yl[vm]
    return y, res


def kernel(**inputs):
    cfg = Cfg()
    y, _ = run(cfg, inputs, trace=False)
    return y
